# revision 29
# baseline (speedup 1.0000x reference)
"""Trainium2 Bass kernel for an AttentionBlock (GroupNorm + 1x1-conv QKV +
4-head attention over 48x48 pixels + 1x1-conv proj + residual).

Contract: kernel(**inputs) takes the FULL unsharded inputs (as produced by
setup_inputs) and returns the FULL output (8, 256, 48, 48) float32.

Strategy: data-parallel over batch — batch element i runs on NeuronCore i.
All parameters are replicated. Per core everything stays on-chip:

  x [256, 2304] (channels on partitions)
  -> GroupNorm via per-channel bn_stats + group-averaging matmul -> affine A,B
  -> q,k = W_qk @ xn  (channels-on-partitions layout, bf16; q pre-scaled 1/8)
  -> vT  = xn^T @ W_v (pixels-on-partitions layout, so PV needs no transpose),
     augmented with a ones column per head (computes softmax denominators for
     free inside the PV matmul)
  -> scores S^T[j,i] = k_j . q_i, with the two heads of a pair issued
     back-to-back on distinct PE row strips (0-63 / 64-127) so they run
     concurrently on the systolic array; exp on ScalarE (no max subtraction;
     scores are small enough for fp32-safe exp), E stored bf16
  -> PV[d,i] = sum_j vT[j,d] E[j,i] accumulated over 18 j-blocks in PSUM;
     softmax division deferred: rows are unnormalized, denominators ride
     along in a spare PSUM row
  -> normalize by broadcasting 1/sums over the 64 rows of each head via a
     tiny selection matmul
  -> proj + residual + (v-bias folded into an effective proj bias on host)
"""

from contextlib import ExitStack

import numpy as np

import concourse.bacc as bacc
import concourse.bass as bass
import concourse.mybir as mybir
import concourse.tile as tile
from concourse.bass_utils import run_bass_kernel_spmd

F32 = mybir.dt.float32
F32R = mybir.dt.float32r
BF16 = mybir.dt.bfloat16
AF = mybir.ActivationFunctionType
OP = mybir.AluOpType

N_CORES = 8
C = 256          # channels
HW = 2304        # 48*48 pixels
NH = 4           # heads
HD = 64          # head dim
G = 32           # groupnorm groups
EPS = 1e-5
CT = 2           # channel partition tiles of 128
PB = 18          # pixel blocks of 128

# pixel chunks for N<=512 matmuls
PCH = [(0, 512), (512, 512), (1024, 512), (1536, 512), (2048, 256)]
# i-chunks for the attention stage (uniform 768 -> 2 PSUM banks per tile)
ICH = [(0, 768), (768, 768), (1536, 768)]

# ones-column position (within each head's 128-col lhsT block) = the PSUM
# partition the softmax denominator lands on. Engine APs must start at a
# partition in {0,32,64,96}; even heads' data sits at partitions 0-63 so their
# denominators go to 64/96, odd heads' data sits at 64-127 so theirs go to 0/32.
ONES_COL = {0: 64, 1: 0, 2: 96, 3: 32}


def _chunks(length):
    out = []
    off = 0
    while off < length:
        cl = min(512, length - off)
        out.append((off, cl))
        off += cl
    return out


def _build():
    nc = bacc.Bacc(
        "TRN2", target_bir_lowering=False, debug=False, num_devices=N_CORES
    )
    x_d = nc.dram_tensor("x", [C, HW], F32, kind="ExternalInput")
    wqkvT_d = nc.dram_tensor("wqkvT", [C, 3 * C], F32R, kind="ExternalInput")
    wprojT_d = nc.dram_tensor("wprojT", [C, C], BF16, kind="ExternalInput")
    gsel_d = nc.dram_tensor("gsel", [C, C], F32, kind="ExternalInput")
    bsel_d = nc.dram_tensor("bsel", [64, C], F32R, kind="ExternalInput")
    # per-channel vectors: [...,0]=gn_w [...,1]=gn_b [...,2]=qb/8 [...,3]=kb
    # [...,4]=proj_b + proj_w @ v_bias
    vecs_d = nc.dram_tensor("vecs", [CT, 128, 5], F32, kind="ExternalInput")
    vmask_d = nc.dram_tensor("vmask", [128, 4 * 128], BF16, kind="ExternalInput")
    out_d = nc.dram_tensor("out", [C, HW], F32, kind="ExternalOutput")

    with ExitStack() as ctx:
        tc = ctx.enter_context(tile.TileContext(nc))
        const = ctx.enter_context(tc.tile_pool(name="const", bufs=1))
        big = ctx.enter_context(tc.tile_pool(name="big", bufs=1))
        xin = ctx.enter_context(tc.tile_pool(name="xin", bufs=2))
        xno = ctx.enter_context(tc.tile_pool(name="xno", bufs=2))
        epool = ctx.enter_context(tc.tile_pool(name="epool", bufs=12))
        small = ctx.enter_context(tc.tile_pool(name="small", bufs=1))
        mmps = ctx.enter_context(
            tc.tile_pool(name="mmps", bufs=2, space=bass.MemorySpace.PSUM)
        )
        pvps = ctx.enter_context(
            tc.tile_pool(name="pvps", bufs=2, space=bass.MemorySpace.PSUM)
        )

        # ---- load inputs ----
        vecs_sb = []
        wqkvT_sb = []
        wprojT_sb = []
        gsel_sb = []
        x_sb = []
        for ct in range(CT):
            xt = xin.tile([128, HW], F32, tag="xin", name=f"x{ct}")
            nc.sync.dma_start(xt[:], x_d[ct * 128 : (ct + 1) * 128, :])
            x_sb.append(xt)
            vt = const.tile([128, 5], F32, tag=f"vecs{ct}")
            nc.sync.dma_start(vt[:], vecs_d[ct])
            vecs_sb.append(vt)
            wq = const.tile([128, 3 * C], F32R, tag=f"wqkv{ct}")
            nc.sync.dma_start(wq[:], wqkvT_d[ct * 128 : (ct + 1) * 128, :])
            wqkvT_sb.append(wq)
            wp = const.tile([128, C], BF16, tag=f"wproj{ct}")
            nc.sync.dma_start(wp[:], wprojT_d[ct * 128 : (ct + 1) * 128, :])
            wprojT_sb.append(wp)
            gs = const.tile([128, C], F32, tag=f"gsel{ct}")
            nc.sync.dma_start(gs[:], gsel_d[ct * 128 : (ct + 1) * 128, :])
            gsel_sb.append(gs)
        bsel_sb = const.tile([64, C], F32R, tag="bsel")
        nc.sync.dma_start(bsel_sb[:], bsel_d[:])
        vmask_sb = const.tile([128, 4 * 128], BF16, tag="vmask")
        nc.sync.dma_start(vmask_sb[:], vmask_d[:])

        # ---- GroupNorm statistics ----
        # per-channel mean/var via bn_stats (9 subgroups of 256), then average
        # groups of 8 channels with the gsel matmul on [mean, E[x^2]]
        stats2 = []
        for ct in range(CT):
            st = small.tile([128, 9, 6], F32, tag=f"bnst{ct}")
            xr = x_sb[ct].rearrange("p (n f) -> p n f", f=256)
            for sg in range(9):
                nc.vector.bn_stats(st[:, sg, :], xr[:, sg, :])
            mv = small.tile([128, 2], F32, tag=f"mv{ct}")
            nc.vector.bn_aggr(mv[:], st[:])
            s2 = small.tile([128, 2], F32, tag=f"s2{ct}")
            nc.vector.tensor_copy(s2[:, 0:1], mv[:, 0:1])
            nc.vector.tensor_tensor(s2[:, 1:2], mv[:, 0:1], mv[:, 0:1], op=OP.mult)
            nc.vector.tensor_add(s2[:, 1:2], s2[:, 1:2], mv[:, 1:2])
            stats2.append(s2)

        eps_sb = small.tile([128, 1], F32, tag="eps")
        nc.vector.memset(eps_sb[:], EPS)
        rsum = small.tile([64, HW], F32R, tag="rsum")
        A_sb = []
        B_sb = []
        for mb in range(CT):
            ps = mmps.tile([128, 2], F32, tag=("stA" if mb % 2 == 0 else "stB"), bufs=1)
            for kt in range(CT):
                nc.tensor.matmul(
                    ps[:],
                    gsel_sb[kt][:, mb * 128 : (mb + 1) * 128],
                    stats2[kt][:],
                    start=(kt == 0),
                    stop=(kt == CT - 1),
                )
            rstd = small.tile([128, 1], F32, tag=f"rstd{mb}")
            msq = small.tile([128, 1], F32, tag=f"msq{mb}")
            mg = small.tile([128, 1], F32, tag=f"mg{mb}")
            nc.vector.tensor_copy(mg[:], ps[:, 0:1])
            nc.vector.tensor_tensor(msq[:], mg[:], mg[:], op=OP.mult)
            nc.vector.tensor_tensor(rstd[:], ps[:, 1:2], msq[:], op=OP.subtract)
            # rstd = 1/sqrt(var + eps)
            nc.scalar.activation(rstd[:], rstd[:], AF.Sqrt, bias=eps_sb[:])
            nc.vector.reciprocal(rstd[:], rstd[:])
            a = small.tile([128, 1], F32, tag=f"A{mb}")
            b = small.tile([128, 1], F32, tag=f"B{mb}")
            nc.vector.tensor_tensor(a[:], vecs_sb[mb][:, 0:1], rstd[:], op=OP.mult)
            nc.vector.tensor_tensor(b[:], mg[:], a[:], op=OP.mult)
            nc.vector.tensor_tensor(b[:], vecs_sb[mb][:, 1:2], b[:], op=OP.subtract)
            A_sb.append(a)
            B_sb.append(b)

        xn_sb = []
        for ct in range(CT):
            xn = xno.tile([128, HW], F32R, tag="xno", name=f"xn{ct}")
            nc.vector.tensor_scalar(
                xn[:], x_sb[ct][:], A_sb[ct][:], B_sb[ct][:], op0=OP.mult, op1=OP.add
            )
            xn_sb.append(xn)

        # ---- qkv: q,k bf16 in channel-layout [256, HW]; v transposed ----
        q_sb = [
            big.tile([128, HW], BF16, tag=f"q{ct}", name=f"q{ct}")
            for ct in range(CT)
        ]
        k_pad = [
            big.tile([128, HW], BF16, tag=f"kp{h}", name=f"kp{h}")
            for h in range(NH)
        ]
        for h in range(NH):
            zro = (1 - h % 2) * 64
            nc.vector.memset(k_pad[h][zro : zro + 64, :], 0.0)
        for which in range(2):  # 0 -> q, 1 -> k
            woff = which * C
            for mb in range(CT):
                for ip, (po, pl) in enumerate(PCH):
                    ps = mmps.tile([128, 1024], F32, tag=("stA" if ip % 2 == 0 else "stB"), bufs=1)
                    for kt in range(CT):
                        nc.tensor.matmul(
                            ps[:, :pl],
                            wqkvT_sb[kt][
                                :, woff + mb * 128 : woff + (mb + 1) * 128
                            ],
                            xn_sb[kt][:, po : po + pl],
                            start=(kt == 0),
                            stop=(kt == CT - 1),
                        )
                    if which == 0:
                        # q = (q_raw + qb) / 8  (qb/8 is precomputed on host)
                        nc.vector.tensor_scalar(
                            q_sb[mb][:, po : po + pl],
                            ps[:, :pl],
                            0.125,
                            vecs_sb[mb][:, 2:3],
                            op0=OP.mult,
                            op1=OP.add,
                        )
                    else:
                        for hh in range(2):
                            h = 2 * mb + hh
                            ro = hh * 64
                            nc.vector.tensor_scalar(
                                k_pad[h][ro : ro + 64, po : po + pl],
                                ps[ro : ro + 64, :pl],
                                vecs_sb[mb][ro : ro + 64, 3:4],
                                None,
                                op0=OP.add,
                            )

        # vT augmented: per j-block a [128, 512] bf16 tile; head h occupies
        # cols h*128..h*128+127 = its 64 v-dims, a ones column, zeros elsewhere
        vtaug = []
        for pb in range(PB):
            vt = big.tile([128, 4 * 128], BF16, tag=f"vt{pb}", name=f"vt{pb}")
            nc.sync.dma_start(vt[:], vmask_sb[:])
            ps = mmps.tile([128, 1024], F32, tag=("stA" if pb % 2 == 0 else "stB"), bufs=1)
            for kt in range(CT):
                nc.tensor.matmul(
                    ps[:, :C],
                    xn_sb[kt][:, pb * 128 : (pb + 1) * 128],
                    wqkvT_sb[kt][:, 2 * C : 3 * C],
                    start=(kt == 0),
                    stop=(kt == CT - 1),
                )
            for h in range(NH):
                dcol = h * 128 + (0 if h % 2 == 0 else 64)
                nc.vector.tensor_copy(
                    vt[:, dcol : dcol + 64], ps[:, h * 64 : (h + 1) * 64]
                )
            vtaug.append(vt)

        # ---- attention: one head PAIR at a time; the two heads' score
        # matmuls go to distinct PE row strips (k/q partition bases 0 vs 64)
        # so the systolic array runs them concurrently ----
        attn_sb = [
            big.tile([128, HW], BF16, tag=f"attn{p}", name=f"attn{p}")
            for p in range(CT)
        ]
        stage = small.tile([128, HW], F32, tag="stage")
        stage2 = small.tile([128, HW], F32, tag="stage2")
        LOOKAHEAD = 3  # PV for j-block jb is emitted after scores of jb+3
        for p in range(CT):
            heads = (2 * p, 2 * p + 1)
            for io, il in ICH:
                es = {h: [None] * PB for h in heads}
                pvs = {}
                for h in heads:
                    pvs[h] = pvps.tile([128, 768], F32, tag="pvps", name=f"pv{h}")

                def emit_pv(pb):
                    for h in heads:
                        for co, cl in _chunks(il):
                            nc.tensor.matmul(
                                pvs[h][:, co : co + cl],
                                vtaug[pb][:, h * 128 : (h + 1) * 128],
                                es[h][pb][:, co : co + cl],
                                start=(pb == 0),
                                stop=(pb == PB - 1),
                            )

                for pb in range(PB):
                    sts = {}
                    for h in heads:
                        sts[h] = mmps.tile(
                            [128, 768],
                            F32,
                            tag=("stA" if h % 2 == 0 else "stB"),
                            bufs=1,
                            name=f"st{h}",
                        )
                    # the two heads go to distinct PE row strips (k/q slices
                    # at partition bases 0 vs 64); adjacent issue -> the
                    # systolic array runs each pair concurrently
                    for co, cl in _chunks(il):
                        for h in heads:
                            nc.tensor.matmul(
                                sts[h][:, co : co + cl],
                                k_pad[h][:, pb * 128 : (pb + 1) * 128],
                                q_sb[p][:, io + co : io + co + cl],
                                start=True,
                                stop=True,
                            )
                    for h in heads:
                        e = epool.tile([128, il], BF16, tag="E", name=f"e{p}_{io}_{pb}")
                        nc.scalar.activation(e[:], sts[h][:, :il], AF.Exp)
                        es[h][pb] = e
                    if pb >= LOOKAHEAD:
                        emit_pv(pb - LOOKAHEAD)
                for pb in range(PB - LOOKAHEAD, PB):
                    emit_pv(pb)
                for h in heads:
                    ro = (h % 2) * 64
                    srow = ONES_COL[h]
                    nc.vector.tensor_copy(
                        attn_sb[p][ro : ro + 64, io : io + il],
                        pvs[h][ro : ro + 64, :il],
                    )
                    nc.vector.tensor_copy(
                        stage[srow : srow + 1, io : io + il],
                        pvs[h][srow : srow + 1, :il],
                    )
                    with nc.allow_low_precision(reason="f32r softmax denom"):
                        nc.vector.reciprocal(
                            stage2[srow : srow + 1, io : io + il].bitcast(F32R),
                            stage[srow : srow + 1, io : io + il],
                        )
                    rrow = 32 * (h // 2) + (h % 2)
                    nc.sync.dma_start(
                        rsum[rrow : rrow + 1, io : io + il],
                        stage2[srow : srow + 1, io : io + il].bitcast(F32R),
                    )

            # normalize this pair's rows (overlaps the next pair's compute)
            for po, pl in PCH:
                rs = mmps.tile([128, 1024], F32, tag=("stA" if p % 2 == 0 else "stB"), bufs=1, name="rs")
                nc.tensor.matmul(
                    rs[:, :pl],
                    bsel_sb[32 * p : 32 * p + 2, p * 128 : (p + 1) * 128],
                    rsum[32 * p : 32 * p + 2, po : po + pl],
                    start=True,
                    stop=True,
                )
                nc.vector.tensor_tensor(
                    attn_sb[p][:, po : po + pl],
                    attn_sb[p][:, po : po + pl],
                    rs[:, :pl],
                    op=OP.mult,
                )

        # ---- tail: proj + residual, pipelined per 512-pixel chunk
        # (x is re-loaded; its tiles were recycled) ----
        x2_sb = []
        ot_sb = []
        for ct in range(CT):
            xt = xin.tile([128, HW], F32, tag="xin", name=f"x2_{ct}")
            nc.sync.dma_start(xt[:], x_d[ct * 128 : (ct + 1) * 128, :])
            x2_sb.append(xt)
            ot = xno.tile([128, HW], F32, tag="xno", name=f"out{ct}")
            nc.vector.tensor_scalar(
                ot[:], xt[:], vecs_sb[ct][:, 4:5], None, op0=OP.add
            )
            ot_sb.append(ot)
        for po, pl in PCH:
            for ct in range(CT):
                ps = mmps.tile([128, 1024], F32, tag=("stA" if ct % 2 == 0 else "stB"), bufs=1, name="prj")
                for kt in range(CT):
                    nc.tensor.matmul(
                        ps[:, :pl],
                        wprojT_sb[kt][:, ct * 128 : (ct + 1) * 128],
                        attn_sb[kt][:, po : po + pl],
                        start=(kt == 0),
                        stop=(kt == CT - 1),
                    )
                nc.vector.tensor_tensor(
                    ot_sb[ct][:, po : po + pl],
                    ot_sb[ct][:, po : po + pl],
                    ps[:, :pl],
                    op=OP.add,
                )
        for ct in range(CT):
            nc.sync.dma_start(out_d[ct * 128 : (ct + 1) * 128, :], ot_sb[ct][:])

    nc.compile()
    return nc


_NC = None


def _get_nc():
    global _NC
    if _NC is None:
        _NC = _build()
    return _NC


def _host_prep(x, gn_w, gn_b, qkv_w, qkv_b, proj_w, proj_b):
    import ml_dtypes

    f32 = np.float32
    x = np.asarray(x, dtype=f32)
    gn_w = np.asarray(gn_w, dtype=f32)
    gn_b = np.asarray(gn_b, dtype=f32)
    qkv_w = np.asarray(qkv_w, dtype=f32)
    qkv_b = np.asarray(qkv_b, dtype=f32)
    proj_w = np.asarray(proj_w, dtype=f32)
    proj_b = np.asarray(proj_b, dtype=f32)

    b = x.shape[0]
    xs = np.ascontiguousarray(x.reshape(b, C, HW))

    wqkvT = np.ascontiguousarray(qkv_w.T)
    wprojT = np.ascontiguousarray(proj_w.T).astype(ml_dtypes.bfloat16)

    gsel = np.zeros((C, C), dtype=f32)
    for g in range(G):
        gsel[g * 8 : (g + 1) * 8, g * 8 : (g + 1) * 8] = 1.0 / 8.0

    bsel = np.zeros((64, C), dtype=f32)
    for p in range(CT):
        bsel[32 * p, p * 128 : p * 128 + 64] = 1.0
        bsel[32 * p + 1, p * 128 + 64 : (p + 1) * 128] = 1.0

    pbeff = proj_b + proj_w @ qkv_b[2 * C : 3 * C]
    vecs = np.stack(
        [gn_w, gn_b, qkv_b[:C] / 8.0, qkv_b[C : 2 * C], pbeff], axis=-1
    ).reshape(CT, 128, 5)
    vecs = np.ascontiguousarray(vecs.astype(f32))

    vmask = np.zeros((128, 4 * 128), dtype=np.float32)
    for h in range(NH):
        vmask[:, h * 128 + ONES_COL[h]] = 1.0
    vmask = vmask.astype(ml_dtypes.bfloat16)

    shared = {
        "wqkvT": wqkvT,
        "wprojT": wprojT,
        "gsel": gsel,
        "bsel": bsel,
        "vecs": vecs,
        "vmask": vmask,
    }
    in_maps = [dict(shared, x=np.ascontiguousarray(xs[i])) for i in range(b)]
    return in_maps, x.shape


def _run(inputs, **run_kwargs):
    nc = _get_nc()
    in_maps, xshape = _host_prep(**inputs)
    res = run_bass_kernel_spmd(
        nc, in_maps, core_ids=list(range(N_CORES)), **run_kwargs
    )
    out = np.stack([res.results[i]["out"] for i in range(N_CORES)])
    return out.reshape(xshape).astype(np.float32), res


def kernel(**inputs):
    out, _ = _run(inputs)
    return out


# revision 30
# speedup vs baseline: 1.1594x; 1.1594x over previous
"""Trainium2 Bass kernel for an AttentionBlock (GroupNorm + 1x1-conv QKV +
4-head attention over 48x48 pixels + 1x1-conv proj + residual).

Contract: kernel(**inputs) takes the FULL unsharded inputs (as produced by
setup_inputs) and returns the FULL output (8, 256, 48, 48) float32.

Strategy: data-parallel over batch — batch element i runs on NeuronCore i.
All parameters are replicated. Per core everything stays on-chip:

  x [256, 2304] (channels on partitions)
  -> GroupNorm via per-channel bn_stats + group-averaging matmul -> affine A,B
  -> q,k = W_qk @ xn  (channels-on-partitions layout, bf16; q pre-scaled 1/8)
  -> vT  = xn^T @ W_v (pixels-on-partitions layout, so PV needs no transpose),
     augmented with a ones column per head (computes softmax denominators for
     free inside the PV matmul)
  -> scores S^T[j,i] = k_j . q_i, with the two heads of a pair issued
     back-to-back on distinct PE row strips (0-63 / 64-127) so they run
     concurrently on the systolic array; exp on ScalarE (no max subtraction;
     scores are small enough for fp32-safe exp), E stored bf16
  -> PV[d,i] = sum_j vT[j,d] E[j,i] accumulated over 18 j-blocks in PSUM;
     softmax division deferred: rows are unnormalized, denominators ride
     along in a spare PSUM row
  -> normalize by broadcasting 1/sums over the 64 rows of each head via a
     tiny selection matmul
  -> proj + residual + (v-bias folded into an effective proj bias on host)
"""

from contextlib import ExitStack

import numpy as np

import concourse.bacc as bacc
import concourse.bass as bass
import concourse.mybir as mybir
import concourse.tile as tile
from concourse.bass_utils import run_bass_kernel_spmd

F32 = mybir.dt.float32
F32R = mybir.dt.float32r
BF16 = mybir.dt.bfloat16
AF = mybir.ActivationFunctionType
OP = mybir.AluOpType

N_CORES = 8
C = 256          # channels
HW = 2304        # 48*48 pixels
NH = 4           # heads
HD = 64          # head dim
G = 32           # groupnorm groups
EPS = 1e-5
CT = 2           # channel partition tiles of 128
PB = 18          # pixel blocks of 128

# pixel chunks for N<=512 matmuls
PCH = [(0, 512), (512, 512), (1024, 512), (1536, 512), (2048, 256)]
# i-chunks for the attention stage (PSUM-bank friendly; small chunk first)
ICH = [(2048, 256), (0, 1024), (1024, 1024)]

# ones-column position (within each head's 128-col lhsT block) = the PSUM
# partition the softmax denominator lands on. Engine APs must start at a
# partition in {0,32,64,96}; even heads' data sits at partitions 0-63 so their
# denominators go to 64/96, odd heads' data sits at 64-127 so theirs go to 0/32.
ONES_COL = {0: 64, 1: 0, 2: 96, 3: 32}


def _chunks(length):
    out = []
    off = 0
    while off < length:
        cl = min(512, length - off)
        out.append((off, cl))
        off += cl
    return out


def _build():
    nc = bacc.Bacc(
        "TRN2", target_bir_lowering=False, debug=False, num_devices=N_CORES
    )
    x_d = nc.dram_tensor("x", [C, HW], F32, kind="ExternalInput")
    wqkvT_d = nc.dram_tensor("wqkvT", [C, 3 * C], F32R, kind="ExternalInput")
    wprojT_d = nc.dram_tensor("wprojT", [C, C], BF16, kind="ExternalInput")
    gsel_d = nc.dram_tensor("gsel", [C, C], F32, kind="ExternalInput")
    bsel_d = nc.dram_tensor("bsel", [64, C], F32R, kind="ExternalInput")
    # per-channel vectors: [...,0]=gn_w [...,1]=gn_b [...,2]=qb/8 [...,3]=kb
    # [...,4]=proj_b + proj_w @ v_bias
    vecs_d = nc.dram_tensor("vecs", [CT, 128, 5], F32, kind="ExternalInput")
    vmask_d = nc.dram_tensor("vmask", [128, 4 * 128], BF16, kind="ExternalInput")
    out_d = nc.dram_tensor("out", [C, HW], F32, kind="ExternalOutput")

    with ExitStack() as ctx:
        tc = ctx.enter_context(tile.TileContext(nc))
        const = ctx.enter_context(tc.tile_pool(name="const", bufs=1))
        big = ctx.enter_context(tc.tile_pool(name="big", bufs=1))
        xin = ctx.enter_context(tc.tile_pool(name="xin", bufs=2))
        xno = ctx.enter_context(tc.tile_pool(name="xno", bufs=2))
        epool = ctx.enter_context(tc.tile_pool(name="epool", bufs=14))
        small = ctx.enter_context(tc.tile_pool(name="small", bufs=1))
        mmps = ctx.enter_context(
            tc.tile_pool(name="mmps", bufs=2, space=bass.MemorySpace.PSUM)
        )
        pvps = ctx.enter_context(
            tc.tile_pool(name="pvps", bufs=2, space=bass.MemorySpace.PSUM)
        )

        # ---- load inputs ----
        vecs_sb = []
        wqkvT_sb = []
        wprojT_sb = []
        gsel_sb = []
        x_sb = []
        for ct in range(CT):
            xt = xin.tile([128, HW], F32, tag="xin", name=f"x{ct}")
            nc.sync.dma_start(xt[:], x_d[ct * 128 : (ct + 1) * 128, :])
            x_sb.append(xt)
            vt = const.tile([128, 5], F32, tag=f"vecs{ct}")
            nc.sync.dma_start(vt[:], vecs_d[ct])
            vecs_sb.append(vt)
            wq = const.tile([128, 3 * C], F32R, tag=f"wqkv{ct}")
            nc.sync.dma_start(wq[:], wqkvT_d[ct * 128 : (ct + 1) * 128, :])
            wqkvT_sb.append(wq)
            wp = const.tile([128, C], BF16, tag=f"wproj{ct}")
            nc.sync.dma_start(wp[:], wprojT_d[ct * 128 : (ct + 1) * 128, :])
            wprojT_sb.append(wp)
            gs = const.tile([128, C], F32, tag=f"gsel{ct}")
            nc.sync.dma_start(gs[:], gsel_d[ct * 128 : (ct + 1) * 128, :])
            gsel_sb.append(gs)
        bsel_sb = const.tile([64, C], F32R, tag="bsel")
        nc.sync.dma_start(bsel_sb[:], bsel_d[:])
        vmask_sb = const.tile([128, 4 * 128], BF16, tag="vmask")
        nc.sync.dma_start(vmask_sb[:], vmask_d[:])

        # ---- GroupNorm statistics ----
        # per-channel mean/var via bn_stats (9 subgroups of 256), then average
        # groups of 8 channels with the gsel matmul on [mean, E[x^2]]
        stats2 = []
        for ct in range(CT):
            st = small.tile([128, 9, 6], F32, tag=f"bnst{ct}")
            xr = x_sb[ct].rearrange("p (n f) -> p n f", f=256)
            for sg in range(9):
                nc.vector.bn_stats(st[:, sg, :], xr[:, sg, :])
            mv = small.tile([128, 2], F32, tag=f"mv{ct}")
            nc.vector.bn_aggr(mv[:], st[:])
            s2 = small.tile([128, 2], F32, tag=f"s2{ct}")
            nc.vector.tensor_copy(s2[:, 0:1], mv[:, 0:1])
            nc.vector.tensor_tensor(s2[:, 1:2], mv[:, 0:1], mv[:, 0:1], op=OP.mult)
            nc.vector.tensor_add(s2[:, 1:2], s2[:, 1:2], mv[:, 1:2])
            stats2.append(s2)

        eps_sb = small.tile([128, 1], F32, tag="eps")
        nc.vector.memset(eps_sb[:], EPS)
        rsum = small.tile([64, HW], F32R, tag="rsum")
        A_sb = []
        B_sb = []
        for mb in range(CT):
            ps = mmps.tile([128, 2], F32, tag=("stA" if mb % 2 == 0 else "stB"), bufs=1)
            for kt in range(CT):
                nc.tensor.matmul(
                    ps[:],
                    gsel_sb[kt][:, mb * 128 : (mb + 1) * 128],
                    stats2[kt][:],
                    start=(kt == 0),
                    stop=(kt == CT - 1),
                )
            rstd = small.tile([128, 1], F32, tag=f"rstd{mb}")
            msq = small.tile([128, 1], F32, tag=f"msq{mb}")
            mg = small.tile([128, 1], F32, tag=f"mg{mb}")
            nc.vector.tensor_copy(mg[:], ps[:, 0:1])
            nc.vector.tensor_tensor(msq[:], mg[:], mg[:], op=OP.mult)
            nc.vector.tensor_tensor(rstd[:], ps[:, 1:2], msq[:], op=OP.subtract)
            # rstd = 1/sqrt(var + eps)
            nc.scalar.activation(rstd[:], rstd[:], AF.Sqrt, bias=eps_sb[:])
            nc.vector.reciprocal(rstd[:], rstd[:])
            a = small.tile([128, 1], F32, tag=f"A{mb}")
            b = small.tile([128, 1], F32, tag=f"B{mb}")
            nc.vector.tensor_tensor(a[:], vecs_sb[mb][:, 0:1], rstd[:], op=OP.mult)
            nc.vector.tensor_tensor(b[:], mg[:], a[:], op=OP.mult)
            nc.vector.tensor_tensor(b[:], vecs_sb[mb][:, 1:2], b[:], op=OP.subtract)
            A_sb.append(a)
            B_sb.append(b)

        xn_sb = []
        for ct in range(CT):
            xn = xno.tile([128, HW], F32R, tag="xno", name=f"xn{ct}")
            nc.vector.tensor_scalar(
                xn[:], x_sb[ct][:], A_sb[ct][:], B_sb[ct][:], op0=OP.mult, op1=OP.add
            )
            xn_sb.append(xn)

        # ---- qkv: q,k bf16 in channel-layout [256, HW]; v transposed ----
        q_sb = [
            big.tile([128, HW], BF16, tag=f"q{ct}", name=f"q{ct}")
            for ct in range(CT)
        ]
        k_pad = [
            big.tile([128, HW], BF16, tag=f"kp{h}", name=f"kp{h}")
            for h in range(NH)
        ]
        for h in range(NH):
            zro = (1 - h % 2) * 64
            nc.vector.memset(k_pad[h][zro : zro + 64, :], 0.0)
        for which in range(2):  # 0 -> q, 1 -> k
            woff = which * C
            for mb in range(CT):
                for ip, (po, pl) in enumerate(PCH):
                    ps = mmps.tile([128, 1024], F32, tag=("stA" if ip % 2 == 0 else "stB"), bufs=1)
                    for kt in range(CT):
                        nc.tensor.matmul(
                            ps[:, :pl],
                            wqkvT_sb[kt][
                                :, woff + mb * 128 : woff + (mb + 1) * 128
                            ],
                            xn_sb[kt][:, po : po + pl],
                            start=(kt == 0),
                            stop=(kt == CT - 1),
                        )
                    if which == 0:
                        # q = (q_raw + qb) / 8  (qb/8 is precomputed on host)
                        nc.vector.tensor_scalar(
                            q_sb[mb][:, po : po + pl],
                            ps[:, :pl],
                            0.125,
                            vecs_sb[mb][:, 2:3],
                            op0=OP.mult,
                            op1=OP.add,
                        )
                    else:
                        for hh in range(2):
                            h = 2 * mb + hh
                            ro = hh * 64
                            nc.vector.tensor_scalar(
                                k_pad[h][ro : ro + 64, po : po + pl],
                                ps[ro : ro + 64, :pl],
                                vecs_sb[mb][ro : ro + 64, 3:4],
                                None,
                                op0=OP.add,
                            )

        # vT augmented: per j-block a [128, 512] bf16 tile; head h occupies
        # cols h*128..h*128+127 = its 64 v-dims, a ones column, zeros elsewhere
        vtaug = []
        for pb in range(PB):
            vt = big.tile([128, 4 * 128], BF16, tag=f"vt{pb}", name=f"vt{pb}")
            nc.sync.dma_start(vt[:], vmask_sb[:])
            ps = mmps.tile([128, 1024], F32, tag=("stA" if pb % 2 == 0 else "stB"), bufs=1)
            for kt in range(CT):
                nc.tensor.matmul(
                    ps[:, :C],
                    xn_sb[kt][:, pb * 128 : (pb + 1) * 128],
                    wqkvT_sb[kt][:, 2 * C : 3 * C],
                    start=(kt == 0),
                    stop=(kt == CT - 1),
                )
            for h in range(NH):
                dcol = h * 128 + (0 if h % 2 == 0 else 64)
                nc.vector.tensor_copy(
                    vt[:, dcol : dcol + 64], ps[:, h * 64 : (h + 1) * 64]
                )
            vtaug.append(vt)

        # ---- attention: one head PAIR at a time; the two heads' score
        # matmuls go to distinct PE row strips (k/q partition bases 0 vs 64)
        # so the systolic array runs them concurrently ----
        attn_sb = [
            big.tile([128, HW], BF16, tag=f"attn{p}", name=f"attn{p}")
            for p in range(CT)
        ]
        stage = small.tile([128, HW], F32, tag="stage")
        stage2 = small.tile([128, HW], F32, tag="stage2")
        LOOKAHEAD = 4  # PV for j-block jb is emitted after scores of jb+4
        for p in range(CT):
            heads = (2 * p, 2 * p + 1)
            for io, il in ICH:
                es = {h: [None] * PB for h in heads}
                pvs = {}
                for h in heads:
                    pvs[h] = pvps.tile([128, 1024], F32, tag="pvps", name=f"pv{h}")

                def emit_pv(pb):
                    for h in heads:
                        for co, cl in _chunks(il):
                            nc.tensor.matmul(
                                pvs[h][:, co : co + cl],
                                vtaug[pb][:, h * 128 : (h + 1) * 128],
                                es[h][pb][:, co : co + cl],
                                start=(pb == 0),
                                stop=(pb == PB - 1),
                            )

                for pb in range(PB):
                    sts = {}
                    for h in heads:
                        sts[h] = mmps.tile(
                            [128, 1024],
                            F32,
                            tag=("stA" if h % 2 == 0 else "stB"),
                            bufs=1,
                            name=f"st{h}",
                        )
                    # the two heads go to distinct PE row strips (k/q slices
                    # at partition bases 0 vs 64); adjacent issue -> the
                    # systolic array runs each pair concurrently
                    for co, cl in _chunks(il):
                        for h in heads:
                            nc.tensor.matmul(
                                sts[h][:, co : co + cl],
                                k_pad[h][:, pb * 128 : (pb + 1) * 128],
                                q_sb[p][:, io + co : io + co + cl],
                                start=True,
                                stop=True,
                            )
                    for h in heads:
                        e = epool.tile([128, il], BF16, tag="E", name=f"e{p}_{io}_{pb}")
                        nc.scalar.activation(e[:], sts[h][:, :il], AF.Exp)
                        es[h][pb] = e
                    if pb >= LOOKAHEAD:
                        emit_pv(pb - LOOKAHEAD)
                for pb in range(PB - LOOKAHEAD, PB):
                    emit_pv(pb)
                for h in heads:
                    ro = (h % 2) * 64
                    srow = ONES_COL[h]
                    nc.vector.tensor_copy(
                        attn_sb[p][ro : ro + 64, io : io + il],
                        pvs[h][ro : ro + 64, :il],
                    )
                    nc.vector.tensor_copy(
                        stage[srow : srow + 1, io : io + il],
                        pvs[h][srow : srow + 1, :il],
                    )
                    with nc.allow_low_precision(reason="f32r softmax denom"):
                        nc.vector.reciprocal(
                            stage2[srow : srow + 1, io : io + il].bitcast(F32R),
                            stage[srow : srow + 1, io : io + il],
                        )
                    rrow = 32 * (h // 2) + (h % 2)
                    nc.sync.dma_start(
                        rsum[rrow : rrow + 1, io : io + il],
                        stage2[srow : srow + 1, io : io + il].bitcast(F32R),
                    )

            # normalize this pair's rows (overlaps the next pair's compute)
            for po, pl in PCH:
                rs = mmps.tile([128, 1024], F32, tag=("stA" if p % 2 == 0 else "stB"), bufs=1, name="rs")
                nc.tensor.matmul(
                    rs[:, :pl],
                    bsel_sb[32 * p : 32 * p + 2, p * 128 : (p + 1) * 128],
                    rsum[32 * p : 32 * p + 2, po : po + pl],
                    start=True,
                    stop=True,
                )
                nc.vector.tensor_tensor(
                    attn_sb[p][:, po : po + pl],
                    attn_sb[p][:, po : po + pl],
                    rs[:, :pl],
                    op=OP.mult,
                )

        # ---- tail: proj + residual, pipelined per 512-pixel chunk
        # (x is re-loaded; its tiles were recycled) ----
        x2_sb = []
        ot_sb = []
        for ct in range(CT):
            xt = xin.tile([128, HW], F32, tag="xin", name=f"x2_{ct}")
            nc.sync.dma_start(xt[:], x_d[ct * 128 : (ct + 1) * 128, :])
            x2_sb.append(xt)
            ot = xno.tile([128, HW], F32, tag="xno", name=f"out{ct}")
            nc.vector.tensor_scalar(
                ot[:], xt[:], vecs_sb[ct][:, 4:5], None, op0=OP.add
            )
            ot_sb.append(ot)
        for po, pl in PCH:
            for ct in range(CT):
                ps = mmps.tile([128, 1024], F32, tag=("stA" if ct % 2 == 0 else "stB"), bufs=1, name="prj")
                for kt in range(CT):
                    nc.tensor.matmul(
                        ps[:, :pl],
                        wprojT_sb[kt][:, ct * 128 : (ct + 1) * 128],
                        attn_sb[kt][:, po : po + pl],
                        start=(kt == 0),
                        stop=(kt == CT - 1),
                    )
                nc.vector.tensor_tensor(
                    ot_sb[ct][:, po : po + pl],
                    ot_sb[ct][:, po : po + pl],
                    ps[:, :pl],
                    op=OP.add,
                )
        for ct in range(CT):
            nc.sync.dma_start(out_d[ct * 128 : (ct + 1) * 128, :], ot_sb[ct][:])

    nc.compile()
    return nc


_NC = None


def _get_nc():
    global _NC
    if _NC is None:
        _NC = _build()
    return _NC


def _host_prep(x, gn_w, gn_b, qkv_w, qkv_b, proj_w, proj_b):
    import ml_dtypes

    f32 = np.float32
    x = np.asarray(x, dtype=f32)
    gn_w = np.asarray(gn_w, dtype=f32)
    gn_b = np.asarray(gn_b, dtype=f32)
    qkv_w = np.asarray(qkv_w, dtype=f32)
    qkv_b = np.asarray(qkv_b, dtype=f32)
    proj_w = np.asarray(proj_w, dtype=f32)
    proj_b = np.asarray(proj_b, dtype=f32)

    b = x.shape[0]
    xs = np.ascontiguousarray(x.reshape(b, C, HW))

    wqkvT = np.ascontiguousarray(qkv_w.T)
    wprojT = np.ascontiguousarray(proj_w.T).astype(ml_dtypes.bfloat16)

    gsel = np.zeros((C, C), dtype=f32)
    for g in range(G):
        gsel[g * 8 : (g + 1) * 8, g * 8 : (g + 1) * 8] = 1.0 / 8.0

    bsel = np.zeros((64, C), dtype=f32)
    for p in range(CT):
        bsel[32 * p, p * 128 : p * 128 + 64] = 1.0
        bsel[32 * p + 1, p * 128 + 64 : (p + 1) * 128] = 1.0

    pbeff = proj_b + proj_w @ qkv_b[2 * C : 3 * C]
    vecs = np.stack(
        [gn_w, gn_b, qkv_b[:C] / 8.0, qkv_b[C : 2 * C], pbeff], axis=-1
    ).reshape(CT, 128, 5)
    vecs = np.ascontiguousarray(vecs.astype(f32))

    vmask = np.zeros((128, 4 * 128), dtype=np.float32)
    for h in range(NH):
        vmask[:, h * 128 + ONES_COL[h]] = 1.0
    vmask = vmask.astype(ml_dtypes.bfloat16)

    shared = {
        "wqkvT": wqkvT,
        "wprojT": wprojT,
        "gsel": gsel,
        "bsel": bsel,
        "vecs": vecs,
        "vmask": vmask,
    }
    in_maps = [dict(shared, x=np.ascontiguousarray(xs[i])) for i in range(b)]
    return in_maps, x.shape


def _run(inputs, **run_kwargs):
    nc = _get_nc()
    in_maps, xshape = _host_prep(**inputs)
    res = run_bass_kernel_spmd(
        nc, in_maps, core_ids=list(range(N_CORES)), **run_kwargs
    )
    out = np.stack([res.results[i]["out"] for i in range(N_CORES)])
    return out.reshape(xshape).astype(np.float32), res


def kernel(**inputs):
    out, _ = _run(inputs)
    return out
